# revision 31
# baseline (speedup 1.0000x reference)
"""DeformableConv1d Trainium2 kernel (8-core data-parallel over batch).

Per batch b, x [C=128, L=16384], all-bf16 matmul pipeline:

  Stage A (offsets), per 2048-col chunk:
    t = y - mean_c(y) = sum_j Mc_j @ x_(j-1),  Mc_j = ((I - J/C) diag(dw_w[:,j]))
    trelu = relu(t + bias_c), tsq = (t + bias_c)^2            (ACT, bias fused)
    st = [off_w | 0; 0 | 1/C] @ [trelu; tsq] in one PSUM bank (PE)
    -> DRAM f32, repacked [C, 4, 8] per half; r = 1/sqrt(s2+eps) (ACT+DVE);
    off = offmm * r (Pool) -> d_off ring [3, DCH] bf16

  Stage B (exact 3-diagonal hat identity, |off| < 1; max|off|=0.67 here):
    dx[m] = x[m+1] - x[m] (Pool); ddx[m] = dx[m+1] - dx[m]     (DVE)
    broadcast off rows to 128 partitions (DMA); alpha = relu(off) (DVE 4x)
    q_ok = off_k (*) dx_(k-1), q_ak = alpha_k (*) ddx_(k-1)
    -- each as ONE batched TT over [C, 3, 1024] per half      (DVE 2x)
    out = sum_k W_k @ (x_k + q_ok + q_ak)  -- 9 bf16 matmuls, one PSUM
    bank per 512-col group; Pool exits PSUM as bf16, host upcasts.

3-deep software pipeline at 2048-col chunk granularity.  The PE queue
is the critical resource (14 matmul streams per 512-col group-slot), so
its in-order queue must never head-of-line block: the st matmuls of
group g are emitted one B-group later than their trelu/tsq producers,
and group 3's st matmul + PSUM exit + h1 stats roundtrip defer to the
NEXT step entirely.  DGE queues are split by producer (SP: x prefetch +
coefficient broadcasts, ACT: stats roundtrip, DVE: off coefficients
out, Pool: output writes) so no DMA waits on a later-emitted producer
in the same queue.
"""

import numpy as np
import ml_dtypes

B, C, L, K = 8, 128, 16384, 3
EPS = 1e-5
NCORES = 8
DCH = 2048            # chunk granularity
NCH = L // DCH
HALF = 1024
BLK = DCH // 128      # packed-smalls cols per partition (16)

_CACHE = {}
LAST_RESULT = None


def _build_nc(n_iters=1):
    import contextlib
    import concourse.bacc as bacc
    import concourse.bass as bass
    import concourse.tile as tile
    from concourse import mybir

    f32 = mybir.dt.float32
    bf16 = mybir.dt.bfloat16
    AF = mybir.ActivationFunctionType
    ALU = mybir.AluOpType

    nc = bacc.Bacc("TRN2", target_bir_lowering=False)

    xbf = nc.declare_dram_parameter("xbf", [C, L + 4], bf16, isOutput=False).ap()
    mw = nc.declare_dram_parameter("mw", [C, K, C], bf16, isOutput=False).ap()
    cwb = nc.declare_dram_parameter("cwb", [C, K, C], bf16, isOutput=False).ap()
    ow8 = nc.declare_dram_parameter("ow8", [C, 8], bf16, isOutput=False).ap()
    biasc = nc.declare_dram_parameter("biasc", [C, 1], f32, isOutput=False).ap()
    out = nc.declare_dram_parameter("out", [C, L], bf16, isOutput=True).ap()


    d_stats = nc.dram_tensor("d_stats", [4, L], f32).ap()
    d_off = [nc.dram_tensor(f"d_off{p}", [K, DCH], bf16).ap() for p in range(3)]

    with tile.TileContext(nc) as tc:
        with contextlib.ExitStack() as ctx:
            res = ctx.enter_context(tc.tile_pool(name="res", bufs=1))
            pxc = ctx.enter_context(tc.tile_pool(name="pxc", bufs=5))
            pdx = ctx.enter_context(tc.tile_pool(name="pdx", bufs=3))
            ptt = ctx.enter_context(tc.tile_pool(name="ptt", bufs=3))
            psm = ctx.enter_context(tc.tile_pool(name="psm", bufs=4))
            pab = ctx.enter_context(tc.tile_pool(name="pab", bufs=3))
            palf = ctx.enter_context(tc.tile_pool(name="palf", bufs=2))
            pq = ctx.enter_context(tc.tile_pool(name="pq", bufs=4))
            psts = ctx.enter_context(tc.tile_pool(name="psts", bufs=2))
            posb = ctx.enter_context(tc.tile_pool(name="posb", bufs=2))
            pt = ctx.enter_context(tc.tile_pool(name="pt", bufs=2, space="PSUM"))
            pst = ctx.enter_context(tc.tile_pool(name="pst", bufs=2, space="PSUM"))
            pout = ctx.enter_context(tc.tile_pool(name="pout", bufs=2, space="PSUM"))

            sb_mw = res.tile([C, K, C], bf16)
            sb_cwb = res.tile([C, K, C], bf16)
            sb_ow8 = res.tile([C, 8], bf16)
            sb_biasc = res.tile([C, 1], f32)
            eps_t = res.tile([C, 1], f32)

            nc.scalar.dma_start(out=sb_mw, in_=mw)
            nc.scalar.dma_start(out=sb_cwb, in_=cwb)
            nc.scalar.dma_start(out=sb_ow8, in_=ow8)
            nc.scalar.dma_start(out=sb_biasc, in_=biasc)
            nc.vector.memset(eps_t, EPS)
            # warm-up read so later ACT ops don't carry the bias-DMA wait
            warm = res.tile([C, 1], f32)
            nc.scalar.activation(out=warm, in_=sb_biasc, func=AF.Copy)

            def t_group(s, sb_xc, g):
                """t matmuls + trelu/tsq for 512-col group g of chunk s.
                The st matmuls are emitted separately (one B-group later)."""
                cb = g * 512
                t_ps = pt.tile([C, 512], f32, tag="t")
                for j in range(K):
                    nc.tensor.matmul(
                        t_ps, sb_mw[:, j, :],
                        sb_xc[:, cb + j + 1 : cb + j + 513],
                        start=(j == 0), stop=(j == K - 1),
                    )
                trelu = ptt.tile([C, 512], bf16, tag="trelu")
                tsq = ptt.tile([C, 512], bf16, tag="tsq")
                nc.scalar.activation(out=trelu, in_=t_ps, func=AF.Relu,
                                     bias=sb_biasc, scale=1.0)
                nc.scalar.activation(out=tsq, in_=t_ps, func=AF.Square,
                                     bias=sb_biasc, scale=1.0)
                return trelu, tsq

            def st_group(s, g, trelu, tsq, st_sb):
                """st matmuls + ACT PSUM exit for group g of chunk s."""
                cb = g * 512
                st_ps = pst.tile([4, 512], f32, tag="st")
                nc.tensor.matmul(st_ps, sb_ow8[:, 0:4], trelu,
                                 start=True, stop=False,
                                 skip_group_check=True)
                nc.tensor.matmul(st_ps, sb_ow8[:, 4:8], tsq,
                                 start=False, stop=True,
                                 skip_group_check=True)
                nc.scalar.activation(out=st_sb[:, cb : cb + 512],
                                     in_=st_ps, func=AF.Copy)

            HBLK = BLK // 2   # packed cols per partition at half granularity

            def smalls_dma(s, st_sb, h):
                """Stats roundtrip (DRAM repack) for half h of chunk s."""
                do = s * DCH
                hb = h * HALF
                nc.sync.dma_start(out=d_stats[:, do + hb : do + hb + HALF],
                                  in_=st_sb[:, hb : hb + HALF])
                packed = psm.tile([C, 4, HBLK], f32, tag="packed")
                nc.sync.dma_start(
                    out=packed,
                    in_=bass.AP(tensor=d_stats.tensor, offset=do + hb,
                                ap=[[HBLK, C], [L, 4], [1, HBLK]]))
                return packed

            def smalls_compute(s, packed, h):
                """rsqrt + off coefficients for half h of chunk s."""
                hb = h * HALF
                rt = psm.tile([C, HBLK], f32, tag="rt")
                nc.scalar.activation(out=rt, in_=packed[:, 3, :],
                                     func=AF.Sqrt, bias=eps_t, scale=1.0)
                nc.vector.reciprocal(out=rt, in_=rt)
                off3 = psm.tile([C, K, HBLK], bf16, tag="off3")
                rtb = bass.AP(tensor=rt.tensor, offset=rt.offset,
                              ap=[rt.ap[0], [0, K], [1, HBLK]])
                nc.gpsimd.tensor_mul(out=off3, in0=packed[:, 0:K, :], in1=rtb)
                nc.sync.dma_start(
                    out=bass.AP(tensor=d_off[s % 3].tensor, offset=hb,
                                ap=[[HBLK, C], [DCH, K], [1, HBLK]]),
                    in_=off3)

            def ab_dma(s, h):
                """Broadcast the 3 off rows of half h to 128 partitions."""
                hb = h * HALF
                ab = pab.tile([C, K, HALF], bf16, tag="ab")
                nc.sync.dma_start(
                    out=ab,
                    in_=bass.AP(tensor=d_off[s % 3].tensor, offset=hb,
                                ap=[[0, C], [DCH, K], [1, HALF]]))
                return ab

            def blend(s, ab, sb_dx, sb_ddx, h):
                """Blend muls for half h of chunk s: q_o = off (*) dx window,
                alpha = relu(off) (4x), q_a = alpha (*) ddx window -- each
                mul ONE batched TT over [C, 3, HALF]."""
                hb = h * HALF
                q = pq.tile([C, 2 * K, HALF], bf16, tag="q")
                dxw = bass.AP(tensor=sb_dx.tensor, offset=sb_dx.offset + hb,
                              ap=[sb_dx.ap[0], [1, K], [1, HALF]])
                nc.vector.tensor_mul(out=q[:, 0:K, :], in0=dxw, in1=ab)
                alf = palf.tile([C, K, HALF], bf16, tag="alf")
                nc.vector.tensor_scalar_max(out=alf, in0=ab, scalar1=0.0)
                ddw = bass.AP(tensor=sb_ddx.tensor, offset=sb_ddx.offset + hb,
                              ap=[sb_ddx.ap[0], [1, K], [1, HALF]])
                nc.vector.tensor_mul(out=q[:, K : 2 * K, :], in0=ddw, in1=alf)
                return q

            def b_group(s, sb_xc, q, g, out_ps):
                """9 accumulating matmuls for 512-col output group g into the
                half-granular PSUM tile (own accumulation group per 512)."""
                cb = g * 512
                gb = (g % 2) * 512
                sl = slice(gb, gb + 512)
                for k in range(K):
                    nc.tensor.matmul(
                        out_ps[:, sl], sb_cwb[:, k, :],
                        sb_xc[:, cb + k + 1 : cb + k + 513],
                        start=(k == 0), stop=False,
                        skip_group_check=True)
                for k in range(K):
                    nc.tensor.matmul(
                        out_ps[:, sl], sb_cwb[:, k, :],
                        q[:, k, gb : gb + 512],
                        start=False, stop=False,
                        skip_group_check=True)
                for k in range(K):
                    nc.tensor.matmul(
                        out_ps[:, sl], sb_cwb[:, k, :],
                        q[:, K + k, gb : gb + 512],
                        start=False, stop=(k == K - 1),
                        skip_group_check=True)

            import contextlib as _ctxlib
            loop_cm = (tc.For_i(0, n_iters, 1) if n_iters > 1
                       else _ctxlib.nullcontext())
            with loop_cm:
                xc_t = {}
                dx_t = {}
                ddx_t = {}
                q_t = {}
                ab_t = {}
                # pend: (chunk, st_sb, trelu3, tsq3) whose g3 st matmul,
                # PSUM exit and whole-chunk stats roundtrip run THIS step
                pend = None
                xc_t[0] = pxc.tile([C, DCH + 4], bf16, tag="xbf", name="xc0")
                nc.sync.dma_start(out=xc_t[0], in_=xbf[:, 0 : DCH + 4])
                for s in range(NCH + 3):
                    run_a = s < NCH
                    run_m = 2 <= s <= NCH + 1      # blend of chunk s-2
                    run_b = s >= 3                 # output of chunk s-3

                    if s + 1 <= NCH - 1:
                        do2 = (s + 1) * DCH
                        xc_t[s + 1] = pxc.tile([C, DCH + 4], bf16, tag="xbf",
                                               name=f"xc{s+1}")
                        nc.sync.dma_start(out=xc_t[s + 1],
                                          in_=xbf[:, do2 : do2 + DCH + 4])

                    # dx for chunk s first on Pool
                    st_sb = None
                    if run_a:
                        st_sb = psts.tile([4, DCH], f32, tag="stsb",
                                          name=f"stsb{s}")
                        ndx = DCH + 3
                        sb_dx = pdx.tile([C, ndx], bf16, tag="dx",
                                         name=f"dx{s}")
                        nc.gpsimd.tensor_sub(
                            out=sb_dx,
                            in0=xc_t[s][:, 1 : 1 + ndx],
                            in1=xc_t[s][:, 0 : ndx])
                        dx_t[s] = sb_dx

                    # blend of chunk s-2, per half, first on DVE: its ab
                    # broadcasts were issued last step, so the muls start
                    # ungated at step start
                    if run_m:
                        for h in range(2):
                            q_t[(s - 2, h)] = blend(
                                s - 2, ab_t.pop((s - 2, h)), dx_t[s - 2],
                                ddx_t[s - 2], h)

                    out_ps = []
                    osb_half = []
                    if run_b:
                        out_ps = [pout.tile([C, HALF], f32, tag="out",
                                            name=f"ops{s}_{h}")
                                  for h in range(2)]
                        osb_half = [posb.tile([C, HALF], bf16, tag=f"osb{h}",
                                              name=f"osb{s}_{h}")
                                    for h in range(2)]

                    # ---- PE slot sequence ----
                    # B0, st(prev,3), t0, B1, st0, t1, B2, st1, t2, B3,
                    # st2, t3; each st matmul trails its trelu/tsq producers
                    # by one B-group so PE's in-order queue never waits on
                    # ACT.  ACT PSUM exits (osb) sit right after the tsq
                    # they can't delay; smalls interleave mid-step.
                    if run_b:
                        b_group(s - 3, xc_t[s - 3], q_t[(s - 3, 0)], 0,
                                out_ps[0])
                    packed_p = None
                    if pend is not None:
                        pc, pst_sb, ptr3, pts3 = pend
                        st_group(pc, 3, ptr3, pts3, pst_sb)
                        packed_p = smalls_dma(pc, pst_sb, 1)
                    tt = {}
                    if run_a:
                        tt[0] = t_group(s, xc_t[s], 0)
                    if run_b:
                        b_group(s - 3, xc_t[s - 3], q_t[(s - 3, 0)], 1,
                                out_ps[0])
                    if run_a:
                        st_group(s, 0, *tt[0], st_sb)
                        tt[1] = t_group(s, xc_t[s], 1)
                    if run_b:
                        # h0 PSUM exit: both its groups stopped, and the
                        # next trelu/tsq PE needs is already queued
                        nc.scalar.activation(out=osb_half[0], in_=out_ps[0],
                                             func=AF.Copy)
                        b_group(s - 3, xc_t[s - 3], q_t[(s - 3, 1)], 2,
                                out_ps[1])
                    packed_h0 = None
                    if run_a:
                        st_group(s, 1, *tt[1], st_sb)
                        packed_h0 = smalls_dma(s, st_sb, 0)
                    if pend is not None:
                        # h1 smalls of the previous chunk: mid-step, after
                        # the trelu/tsq the PE needs soonest.  With d_off(pc)
                        # now fully published, issue its broadcasts so next
                        # step's blend starts ungated.
                        smalls_compute(pc, packed_p, 1)
                        for h in range(2):
                            ab_t[(pc, h)] = ab_dma(pc, h)
                    if run_a:
                        tt[2] = t_group(s, xc_t[s], 2)
                    if run_b:
                        b_group(s - 3, xc_t[s - 3], q_t[(s - 3, 1)], 3,
                                out_ps[1])
                    if run_a:
                        st_group(s, 2, *tt[2], st_sb)
                        smalls_compute(s, packed_h0, 0)
                        tt[3] = t_group(s, xc_t[s], 3)
                        # ddx late on DVE: consumed by the blend two steps
                        # out, never head-of-line blocks this step's q muls
                        ndx = DCH + 3
                        sb_ddx = pdx.tile([C, ndx - 1], bf16, tag="ddx",
                                          name=f"ddx{s}")
                        nc.vector.tensor_sub(
                            out=sb_ddx,
                            in0=dx_t[s][:, 1:ndx],
                            in1=dx_t[s][:, 0 : ndx - 1])
                        ddx_t[s] = sb_ddx
                        pend = (s, st_sb, tt[3][0], tt[3][1])
                    else:
                        pend = None
                    if run_b:
                        # h1 PSUM exit + output DMAs at step end on the ACT
                        # HWDGE queue, behind their producers
                        nc.scalar.activation(out=osb_half[1], in_=out_ps[1],
                                             func=AF.Copy)
                        for h in range(2):
                            o = (s - 3) * DCH + h * HALF
                            nc.scalar.dma_start(out=out[:, o : o + HALF],
                                                in_=osb_half[h])
                        del xc_t[s - 3], q_t[(s - 3, 0)], q_t[(s - 3, 1)]
                    if run_m:
                        del dx_t[s - 2], ddx_t[s - 2]

    nc.compile()
    return nc


def _host_prep(inputs):
    x = np.ascontiguousarray(inputs["x"], np.float32)
    dw_w = np.asarray(inputs["dw_w"], np.float32)
    dw_b = np.asarray(inputs["dw_b"], np.float32)
    ln_g = np.asarray(inputs["ln_g"], np.float32)
    ln_b = np.asarray(inputs["ln_b"], np.float32)
    off_w = np.asarray(inputs["off_w"], np.float32)
    off_b = np.asarray(inputs["off_b"], np.float32)
    dc_w = np.asarray(inputs["dc_w"], np.float32)
    assert np.all(ln_g == 1.0) and np.all(ln_b == 0.0) and np.all(off_b == 0.0)
    bf = ml_dtypes.bfloat16

    w = dw_w[:, 0, :]                       # [C, K]
    cen = np.eye(C) - 1.0 / C
    mw = np.stack([(cen @ np.diag(w[:, j])).T for j in range(K)], axis=1).astype(bf)
    biasc = (dw_b - dw_b.mean())[:, None].astype(np.float32)
    cw = np.stack([dc_w[:, :, k].T for k in range(K)], axis=1)   # [c, k, o]
    cwb = np.ascontiguousarray(cw).astype(bf)
    ow8 = np.zeros((C, 8), np.float32)
    ow8[:, 0:3] = off_w.T
    ow8[:, 7] = 1.0 / C
    ow8 = ow8.astype(bf)

    xbfp = np.zeros((B, C, L + 4), bf)
    xbfp[:, :, 2 : 2 + L] = x.astype(bf)

    return [dict(xbf=xbfp[b], mw=mw, cwb=cwb, ow8=ow8, biasc=biasc)
            for b in range(B)]


def kernel(**inputs):
    global LAST_RESULT
    from concourse.bass_utils import run_bass_kernel_spmd

    if "nc" not in _CACHE:
        _CACHE["nc"] = _build_nc()
    nc = _CACHE["nc"]
    in_maps = _host_prep(inputs)
    res = run_bass_kernel_spmd(nc, in_maps, list(range(NCORES)))
    LAST_RESULT = res
    out = np.stack([np.asarray(res.results[i]["out"]).astype(np.float32)
                    for i in range(NCORES)])
    return out


# revision 35
# speedup vs baseline: 1.0236x; 1.0236x over previous
"""DeformableConv1d Trainium2 kernel (8-core data-parallel over batch).

Per batch b, x [C=128, L=16384], all-bf16 matmul pipeline:

  Stage A (offsets), per 2048-col chunk:
    t = y - mean_c(y) = sum_j Mc_j @ x_(j-1),  Mc_j = ((I - J/C) diag(dw_w[:,j]))
    trelu = relu(t + bias_c), tsq = (t + bias_c)^2            (ACT, bias fused)
    st = [off_w | 0; 0 | 1/C] @ [trelu; tsq] in one PSUM bank (PE)
    -> DRAM f32, repacked [C, 4, 8] per half; r = 1/sqrt(s2+eps) (ACT+DVE);
    off = offmm * r (Pool) -> d_off ring [3, DCH] bf16

  Stage B (exact 3-diagonal hat identity, |off| < 1; max|off|=0.67 here):
    dx[m] = x[m+1] - x[m] (Pool); ddx[m] = dx[m+1] - dx[m]     (DVE)
    broadcast off rows to 128 partitions (DMA); alpha = relu(off) (DVE 4x)
    q_ok = off_k (*) dx_(k-1), q_ak = alpha_k (*) ddx_(k-1)
    -- each as ONE batched TT over [C, 3, 1024] per half      (DVE 2x)
    out = sum_k W_k @ (x_k + q_ok + q_ak)  -- 9 bf16 matmuls, one PSUM
    bank per 512-col group; Pool exits PSUM as bf16, host upcasts.

3-deep software pipeline at 2048-col chunk granularity.  The PE queue
is the critical resource (14 matmul streams per 512-col group-slot), so
its in-order queue must never head-of-line block: the st matmuls of
group g are emitted one B-group later than their trelu/tsq producers,
and group 3's st matmul + PSUM exit + h1 stats roundtrip defer to the
NEXT step entirely.  DGE queues are split by producer (SP: x prefetch +
coefficient broadcasts, ACT: stats roundtrip, DVE: off coefficients
out, Pool: output writes) so no DMA waits on a later-emitted producer
in the same queue.
"""

import numpy as np
import ml_dtypes

B, C, L, K = 8, 128, 16384, 3
EPS = 1e-5
NCORES = 8
DCH = 2048            # chunk granularity
NCH = L // DCH
HALF = 1024
BLK = DCH // 128      # packed-smalls cols per partition (16)

_CACHE = {}
LAST_RESULT = None


def _build_nc(n_iters=1):
    import contextlib
    import concourse.bacc as bacc
    import concourse.bass as bass
    import concourse.tile as tile
    from concourse import mybir

    f32 = mybir.dt.float32
    bf16 = mybir.dt.bfloat16
    AF = mybir.ActivationFunctionType
    ALU = mybir.AluOpType

    nc = bacc.Bacc("TRN2", target_bir_lowering=False)

    xbf = nc.declare_dram_parameter("xbf", [C, L + 4], bf16, isOutput=False).ap()
    mw = nc.declare_dram_parameter("mw", [C, K, C], bf16, isOutput=False).ap()
    cwb = nc.declare_dram_parameter("cwb", [C, K, C], bf16, isOutput=False).ap()
    ow8 = nc.declare_dram_parameter("ow8", [C, 8], bf16, isOutput=False).ap()
    biasc = nc.declare_dram_parameter("biasc", [C, 1], f32, isOutput=False).ap()
    out = nc.declare_dram_parameter("out", [C, L], bf16, isOutput=True).ap()


    d_stats = nc.dram_tensor("d_stats", [4, L], f32).ap()
    d_off = [nc.dram_tensor(f"d_off{p}", [K, DCH], bf16).ap() for p in range(3)]

    with tile.TileContext(nc) as tc:
        with contextlib.ExitStack() as ctx:
            res = ctx.enter_context(tc.tile_pool(name="res", bufs=1))
            pxc = ctx.enter_context(tc.tile_pool(name="pxc", bufs=6))
            pdx = ctx.enter_context(tc.tile_pool(name="pdx", bufs=4))
            ptt = ctx.enter_context(tc.tile_pool(name="ptt", bufs=4))
            psm = ctx.enter_context(tc.tile_pool(name="psm", bufs=6))
            pab = ctx.enter_context(tc.tile_pool(name="pab", bufs=4))
            palf = ctx.enter_context(tc.tile_pool(name="palf", bufs=3))
            pq = ctx.enter_context(tc.tile_pool(name="pq", bufs=5))
            psts = ctx.enter_context(tc.tile_pool(name="psts", bufs=3))
            posb = ctx.enter_context(tc.tile_pool(name="posb", bufs=3))
            pt = ctx.enter_context(tc.tile_pool(name="pt", bufs=2, space="PSUM"))
            pst = ctx.enter_context(tc.tile_pool(name="pst", bufs=2, space="PSUM"))
            pout = ctx.enter_context(tc.tile_pool(name="pout", bufs=2, space="PSUM"))

            sb_mw = res.tile([C, K, C], bf16)
            sb_cwb = res.tile([C, K, C], bf16)
            sb_ow8 = res.tile([C, 8], bf16)
            sb_biasc = res.tile([C, 1], f32)
            eps_t = res.tile([C, 1], f32)

            nc.scalar.dma_start(out=sb_mw, in_=mw)
            nc.scalar.dma_start(out=sb_cwb, in_=cwb)
            nc.scalar.dma_start(out=sb_ow8, in_=ow8)
            nc.scalar.dma_start(out=sb_biasc, in_=biasc)
            nc.vector.memset(eps_t, EPS)
            # warm-up read so later ACT ops don't carry the bias-DMA wait
            warm = res.tile([C, 1], f32)
            nc.scalar.activation(out=warm, in_=sb_biasc, func=AF.Copy)

            def t_group(s, sb_xc, g):
                """t matmuls + trelu/tsq for 512-col group g of chunk s.
                The st matmuls are emitted separately (one B-group later)."""
                cb = g * 512
                t_ps = pt.tile([C, 512], f32, tag="t")
                for j in range(K):
                    nc.tensor.matmul(
                        t_ps, sb_mw[:, j, :],
                        sb_xc[:, cb + j + 1 : cb + j + 513],
                        start=(j == 0), stop=(j == K - 1),
                    )
                trelu = ptt.tile([C, 512], bf16, tag="trelu")
                tsq = ptt.tile([C, 512], bf16, tag="tsq")
                nc.scalar.activation(out=trelu, in_=t_ps, func=AF.Relu,
                                     bias=sb_biasc, scale=1.0)
                nc.scalar.activation(out=tsq, in_=t_ps, func=AF.Square,
                                     bias=sb_biasc, scale=1.0)
                return trelu, tsq

            def st_group(s, g, trelu, tsq, st_sb):
                """st matmuls + ACT PSUM exit for group g of chunk s."""
                cb = g * 512
                st_ps = pst.tile([4, 512], f32, tag="st")
                nc.tensor.matmul(st_ps, sb_ow8[:, 0:4], trelu,
                                 start=True, stop=False,
                                 skip_group_check=True)
                nc.tensor.matmul(st_ps, sb_ow8[:, 4:8], tsq,
                                 start=False, stop=True,
                                 skip_group_check=True)
                nc.scalar.activation(out=st_sb[:, cb : cb + 512],
                                     in_=st_ps, func=AF.Copy)

            HBLK = BLK // 2   # packed cols per partition at half granularity

            def smalls_dma(s, st_sb, h):
                """Stats roundtrip (DRAM repack) for half h of chunk s."""
                do = s * DCH
                hb = h * HALF
                nc.sync.dma_start(out=d_stats[:, do + hb : do + hb + HALF],
                                  in_=st_sb[:, hb : hb + HALF])
                packed = psm.tile([C, 4, HBLK], f32, tag="packed")
                nc.sync.dma_start(
                    out=packed,
                    in_=bass.AP(tensor=d_stats.tensor, offset=do + hb,
                                ap=[[HBLK, C], [L, 4], [1, HBLK]]))
                return packed

            def smalls_compute(s, packed, h):
                """rsqrt + off coefficients for half h of chunk s."""
                hb = h * HALF
                rt = psm.tile([C, HBLK], f32, tag="rt")
                nc.scalar.activation(out=rt, in_=packed[:, 3, :],
                                     func=AF.Sqrt, bias=eps_t, scale=1.0)
                nc.vector.reciprocal(out=rt, in_=rt)
                off3 = psm.tile([C, K, HBLK], bf16, tag="off3")
                rtb = bass.AP(tensor=rt.tensor, offset=rt.offset,
                              ap=[rt.ap[0], [0, K], [1, HBLK]])
                nc.gpsimd.tensor_mul(out=off3, in0=packed[:, 0:K, :], in1=rtb)
                nc.sync.dma_start(
                    out=bass.AP(tensor=d_off[s % 3].tensor, offset=hb,
                                ap=[[HBLK, C], [DCH, K], [1, HBLK]]),
                    in_=off3)

            def ab_dma(s, h):
                """Broadcast the 3 off rows of half h to 128 partitions."""
                hb = h * HALF
                ab = pab.tile([C, K, HALF], bf16, tag="ab")
                nc.sync.dma_start(
                    out=ab,
                    in_=bass.AP(tensor=d_off[s % 3].tensor, offset=hb,
                                ap=[[0, C], [DCH, K], [1, HALF]]))
                return ab

            def blend(s, ab, sb_dx, sb_ddx, h):
                """Blend muls for half h of chunk s: q_o = off (*) dx window,
                alpha = relu(off) (4x), q_a = alpha (*) ddx window -- each
                mul ONE batched TT over [C, 3, HALF]."""
                hb = h * HALF
                q = pq.tile([C, 2 * K, HALF], bf16, tag="q")
                dxw = bass.AP(tensor=sb_dx.tensor, offset=sb_dx.offset + hb,
                              ap=[sb_dx.ap[0], [1, K], [1, HALF]])
                nc.vector.tensor_mul(out=q[:, 0:K, :], in0=dxw, in1=ab)
                alf = palf.tile([C, K, HALF], bf16, tag="alf")
                nc.vector.tensor_scalar_max(out=alf, in0=ab, scalar1=0.0)
                ddw = bass.AP(tensor=sb_ddx.tensor, offset=sb_ddx.offset + hb,
                              ap=[sb_ddx.ap[0], [1, K], [1, HALF]])
                nc.vector.tensor_mul(out=q[:, K : 2 * K, :], in0=ddw, in1=alf)
                return q

            def b_group(s, sb_xc, q, g, out_ps):
                """9 accumulating matmuls for 512-col output group g into the
                half-granular PSUM tile (own accumulation group per 512)."""
                cb = g * 512
                gb = (g % 2) * 512
                sl = slice(gb, gb + 512)
                for k in range(K):
                    nc.tensor.matmul(
                        out_ps[:, sl], sb_cwb[:, k, :],
                        sb_xc[:, cb + k + 1 : cb + k + 513],
                        start=(k == 0), stop=False,
                        skip_group_check=True)
                for k in range(K):
                    nc.tensor.matmul(
                        out_ps[:, sl], sb_cwb[:, k, :],
                        q[:, k, gb : gb + 512],
                        start=False, stop=False,
                        skip_group_check=True)
                for k in range(K):
                    nc.tensor.matmul(
                        out_ps[:, sl], sb_cwb[:, k, :],
                        q[:, K + k, gb : gb + 512],
                        start=False, stop=(k == K - 1),
                        skip_group_check=True)

            import contextlib as _ctxlib
            loop_cm = (tc.For_i(0, n_iters, 1) if n_iters > 1
                       else _ctxlib.nullcontext())
            with loop_cm:
                xc_t = {}
                dx_t = {}
                ddx_t = {}
                q_t = {}
                ab_t = {}
                # pend: (chunk, st_sb, trelu3, tsq3) whose g3 st matmul,
                # PSUM exit and whole-chunk stats roundtrip run THIS step
                pend = None
                xc_t[0] = pxc.tile([C, DCH + 4], bf16, tag="xbf", name="xc0")
                nc.sync.dma_start(out=xc_t[0], in_=xbf[:, 0 : DCH + 4])
                for s in range(NCH + 3):
                    run_a = s < NCH
                    run_m = 2 <= s <= NCH + 1      # blend of chunk s-2
                    run_b = s >= 3                 # output of chunk s-3

                    if s + 1 <= NCH - 1:
                        do2 = (s + 1) * DCH
                        xc_t[s + 1] = pxc.tile([C, DCH + 4], bf16, tag="xbf",
                                               name=f"xc{s+1}")
                        nc.sync.dma_start(out=xc_t[s + 1],
                                          in_=xbf[:, do2 : do2 + DCH + 4])

                    # dx for chunk s first on Pool
                    st_sb = None
                    if run_a:
                        st_sb = psts.tile([4, DCH], f32, tag="stsb",
                                          name=f"stsb{s}")
                        ndx = DCH + 3
                        sb_dx = pdx.tile([C, ndx], bf16, tag="dx",
                                         name=f"dx{s}")
                        nc.gpsimd.tensor_sub(
                            out=sb_dx,
                            in0=xc_t[s][:, 1 : 1 + ndx],
                            in1=xc_t[s][:, 0 : ndx])
                        dx_t[s] = sb_dx

                    # blend of chunk s-2, per half, first on DVE: its ab
                    # broadcasts were issued last step, so the muls start
                    # ungated at step start
                    if run_m:
                        for h in range(2):
                            q_t[(s - 2, h)] = blend(
                                s - 2, ab_t.pop((s - 2, h)), dx_t[s - 2],
                                ddx_t[s - 2], h)

                    out_ps = []
                    osb_half = []
                    if run_b:
                        out_ps = [pout.tile([C, HALF], f32, tag="out",
                                            name=f"ops{s}_{h}")
                                  for h in range(2)]
                        osb_half = [posb.tile([C, HALF], bf16, tag=f"osb{h}",
                                              name=f"osb{s}_{h}")
                                    for h in range(2)]

                    # ---- PE slot sequence ----
                    # B0, st(prev,3), t0, B1, st0, t1, B2, st1, t2, B3,
                    # st2, t3; each st matmul trails its trelu/tsq producers
                    # by one B-group so PE's in-order queue never waits on
                    # ACT.  ACT PSUM exits (osb) sit right after the tsq
                    # they can't delay; smalls interleave mid-step.
                    if run_b:
                        b_group(s - 3, xc_t[s - 3], q_t[(s - 3, 0)], 0,
                                out_ps[0])
                    packed_p = None
                    if pend is not None:
                        pc, pst_sb, ptr3, pts3 = pend
                        st_group(pc, 3, ptr3, pts3, pst_sb)
                        packed_p = smalls_dma(pc, pst_sb, 1)
                    tt = {}
                    if run_a:
                        tt[0] = t_group(s, xc_t[s], 0)
                    if run_b:
                        b_group(s - 3, xc_t[s - 3], q_t[(s - 3, 0)], 1,
                                out_ps[0])
                    if run_a:
                        st_group(s, 0, *tt[0], st_sb)
                        tt[1] = t_group(s, xc_t[s], 1)
                    if run_b:
                        # h0 PSUM exit: both its groups stopped, and the
                        # next trelu/tsq PE needs is already queued
                        nc.scalar.activation(out=osb_half[0], in_=out_ps[0],
                                             func=AF.Copy)
                        b_group(s - 3, xc_t[s - 3], q_t[(s - 3, 1)], 2,
                                out_ps[1])
                    packed_h0 = None
                    if run_a:
                        st_group(s, 1, *tt[1], st_sb)
                        packed_h0 = smalls_dma(s, st_sb, 0)
                    if pend is not None:
                        # h1 smalls of the previous chunk: mid-step, after
                        # the trelu/tsq the PE needs soonest.  With d_off(pc)
                        # now fully published, issue its broadcasts so next
                        # step's blend starts ungated.
                        smalls_compute(pc, packed_p, 1)
                        for h in range(2):
                            ab_t[(pc, h)] = ab_dma(pc, h)
                    if run_a:
                        tt[2] = t_group(s, xc_t[s], 2)
                    if run_b:
                        b_group(s - 3, xc_t[s - 3], q_t[(s - 3, 1)], 3,
                                out_ps[1])
                    if run_a:
                        st_group(s, 2, *tt[2], st_sb)
                        smalls_compute(s, packed_h0, 0)
                        tt[3] = t_group(s, xc_t[s], 3)
                        # ddx late on DVE: consumed by the blend two steps
                        # out, never head-of-line blocks this step's q muls
                        ndx = DCH + 3
                        sb_ddx = pdx.tile([C, ndx - 1], bf16, tag="ddx",
                                          name=f"ddx{s}")
                        nc.vector.tensor_sub(
                            out=sb_ddx,
                            in0=dx_t[s][:, 1:ndx],
                            in1=dx_t[s][:, 0 : ndx - 1])
                        ddx_t[s] = sb_ddx
                        pend = (s, st_sb, tt[3][0], tt[3][1])
                    else:
                        pend = None
                    if run_b:
                        # h1 PSUM exit + output DMAs at step end on the ACT
                        # HWDGE queue, behind their producers
                        nc.scalar.activation(out=osb_half[1], in_=out_ps[1],
                                             func=AF.Copy)
                        for h in range(2):
                            o = (s - 3) * DCH + h * HALF
                            nc.scalar.dma_start(out=out[:, o : o + HALF],
                                                in_=osb_half[h])
                        del xc_t[s - 3], q_t[(s - 3, 0)], q_t[(s - 3, 1)]
                    if run_m:
                        del dx_t[s - 2], ddx_t[s - 2]

    nc.compile()
    return nc


def _host_prep(inputs):
    x = np.ascontiguousarray(inputs["x"], np.float32)
    dw_w = np.asarray(inputs["dw_w"], np.float32)
    dw_b = np.asarray(inputs["dw_b"], np.float32)
    ln_g = np.asarray(inputs["ln_g"], np.float32)
    ln_b = np.asarray(inputs["ln_b"], np.float32)
    off_w = np.asarray(inputs["off_w"], np.float32)
    off_b = np.asarray(inputs["off_b"], np.float32)
    dc_w = np.asarray(inputs["dc_w"], np.float32)
    assert np.all(ln_g == 1.0) and np.all(ln_b == 0.0) and np.all(off_b == 0.0)
    bf = ml_dtypes.bfloat16

    w = dw_w[:, 0, :]                       # [C, K]
    cen = np.eye(C) - 1.0 / C
    mw = np.stack([(cen @ np.diag(w[:, j])).T for j in range(K)], axis=1).astype(bf)
    biasc = (dw_b - dw_b.mean())[:, None].astype(np.float32)
    cw = np.stack([dc_w[:, :, k].T for k in range(K)], axis=1)   # [c, k, o]
    cwb = np.ascontiguousarray(cw).astype(bf)
    ow8 = np.zeros((C, 8), np.float32)
    ow8[:, 0:3] = off_w.T
    ow8[:, 7] = 1.0 / C
    ow8 = ow8.astype(bf)

    xbfp = np.zeros((B, C, L + 4), bf)
    xbfp[:, :, 2 : 2 + L] = x.astype(bf)

    return [dict(xbf=xbfp[b], mw=mw, cwb=cwb, ow8=ow8, biasc=biasc)
            for b in range(B)]


def kernel(**inputs):
    global LAST_RESULT
    from concourse.bass_utils import run_bass_kernel_spmd

    if "nc" not in _CACHE:
        _CACHE["nc"] = _build_nc()
    nc = _CACHE["nc"]
    in_maps = _host_prep(inputs)
    res = run_bass_kernel_spmd(nc, in_maps, list(range(NCORES)))
    LAST_RESULT = res
    out = np.stack([np.asarray(res.results[i]["out"]).astype(np.float32)
                    for i in range(NCORES)])
    return out
